# revision 1
# baseline (speedup 1.0000x reference)
"""CQAttention Trainium2 kernel — data-parallel over batch across 8 NeuronCores.

Problem shapes (hardcoded): B=32, H=256, Lc=1024, Lq=256.
Each core processes B/8 = 4 batches.

Math (per batch, with all-ones masks — guaranteed by the problem spec):
  Ct = C^T [Lc,H], Qt = Q^T [Lq,H]
  S[l,m] = Ct[l]@w1 + Qt[m]@w2 + (Ct[l]*w3)@Qt[m]
  Z = exp(S + r[l] + q[m]) serves BOTH softmaxes:
    S_row = Z / rowsum(Z)   (row term r cancels in row softmax)
    S_col = Z / colsum(Z)   (col term q cancels in col softmax)
  A  = S_row @ Qt
  Bv = S_row @ (S_col^T @ Ct)      (factored: avoids the Lc x Lc product)
  out = relu([Ct, A, Ct*A, Ct*Bv] @ W_res^T + b_res)^T  -> [H, Lc]

Implementation notes:
  - The S (logit) matmuls run in fp32r: full PE rate (1 cycle/row for
    N>=256) with near-fp32 accuracy; plain fp32 matmuls are 4x slower.
  - Everything downstream of exp (attention weights in [0,1], T, A, Bv,
    final projection) runs in bf16 (rel err ~3e-3 total, gate is 2e-2).
  - Transposes (W^T once, C^T/Q^T per batch) are PE transposes of the
    bf16 copies (1 cycle/row). DMA-xbar transposes were tried and are
    both hazardous (xbar output corrupts when DmaTranspose shares a
    HWDGE queue with DMACopy — this Tile version does not serialize
    them) and slower end-to-end on every queue arrangement tested.
  - exp's accum_out produces the row/col softmax sums for free; the bias
    terms r[l], q[m] are folded into the stationary matmul operands
    (CA = C*w3+w2, QA = Q*w3+w1) and the exp's per-partition bias
    (r_col/q_col via a small DRAM-bounce layout shuffle).
  - Emission is software-pipelined: frontend(b+1) (DMA loads, bf16
    casts, transposes, CA/QA, r/q) is emitted before backend(b), which
    removed ~60us of PE idle at batch boundaries.
"""

import numpy as np

_CACHE = {}

B_FULL = 32
N_CORES = 8
BB = B_FULL // N_CORES  # batches per core = 4
H = 256
LC = 1024
LQ = 256


def _build(reps: int = 1):
    from contextlib import ExitStack

    import concourse.bass as bass
    import concourse.tile as tile
    from concourse import bacc, mybir
    from concourse.masks import make_identity

    f32 = mybir.dt.float32
    f32r = mybir.dt.float32r
    bf16 = mybir.dt.bfloat16
    AF = mybir.ActivationFunctionType
    OP = mybir.AluOpType

    nc = bacc.Bacc("TRN2", target_bir_lowering=False, debug=False)

    def mm(out, lhsT, rhs, start, stop):
        # fp32r runs the PE at full rate (1 cycle/row for N>=256) vs 4x for fp32
        nc.tensor.matmul(
            out,
            lhsT=lhsT.bitcast(f32r),
            rhs=rhs.bitcast(f32r),
            start=start,
            stop=stop,
        )

    def mmb(out, lhsT, rhs, start, stop):
        nc.tensor.matmul(out, lhsT=lhsT, rhs=rhs, start=start, stop=stop)

    C = nc.dram_tensor("C", [BB, H, LC], f32, kind="ExternalInput")
    Q = nc.dram_tensor("Q", [BB, H, LQ], f32, kind="ExternalInput")
    w = nc.dram_tensor("w", [3 * H], f32, kind="ExternalInput")
    W_res = nc.dram_tensor("W_res", [H, 4 * H], f32, kind="ExternalInput")
    b_res = nc.dram_tensor("b_res", [H], f32, kind="ExternalInput")
    out = nc.dram_tensor("out", [BB, H, LC], f32, kind="ExternalOutput")

    KH = H // 128  # 2 h-chunks
    NLT = LC // 128  # 8 l-tiles
    NMT = LQ // 128  # 2 m-tiles

    with tile.TileContext(nc) as tc:
        with ExitStack() as ctx:
            singles = ctx.enter_context(tc.tile_pool(name="singles", bufs=1))
            sb = ctx.enter_context(tc.tile_pool(name="sb", bufs=2))
            sb1 = ctx.enter_context(tc.tile_pool(name="sb1", bufs=3))
            sbig = ctx.enter_context(tc.tile_pool(name="sbig", bufs=2))
            sbig1 = ctx.enter_context(tc.tile_pool(name="sbig1", bufs=3))
            ps_tr = ctx.enter_context(
                tc.tile_pool(name="ps_tr", bufs=2, space="PSUM")
            )
            ps_z = ctx.enter_context(
                tc.tile_pool(name="ps_z", bufs=2, space="PSUM")
            )
            ps_big = ctx.enter_context(
                tc.tile_pool(name="ps_big", bufs=2, space="PSUM")
            )
            dr = ctx.enter_context(tc.tile_pool(name="dr", bufs=2, space="DRAM"))

            # ---- one-time constants ----
            identity_bf = singles.tile([128, 128], bf16)
            make_identity(nc, identity_bf)

            w1_col = singles.tile([128, KH], f32r)
            w2_col = singles.tile([128, KH], f32r)
            w3_col = singles.tile([128, KH], f32)
            nc.sync.dma_start(
                out=w1_col,
                in_=w.ap()[0:H].rearrange("(i p) -> p i", i=KH, p=128).bitcast(f32r),
            )
            nc.sync.dma_start(
                out=w2_col,
                in_=w.ap()[H : 2 * H]
                .rearrange("(i p) -> p i", i=KH, p=128)
                .bitcast(f32r),
            )
            nc.sync.dma_start(
                out=w3_col,
                in_=w.ap()[2 * H : 3 * H].rearrange("(i p) -> p i", i=KH, p=128),
            )
            b_col = singles.tile([128, KH], f32)
            nc.sync.dma_start(
                out=b_col, in_=b_res.ap().rearrange("(i p) -> p i", i=KH, p=128)
            )

            # W_res^T (bf16): WT[f][p, ho] = W_res[ho, 128*f + p]
            WT = []
            for f in range(8):
                t_wt = singles.tile([128, H], bf16, tag=f"wt{f}")
                WT.append(t_wt)
            for j in range(KH):
                t = singles.tile([128, 4 * H], f32, tag=f"wn{j}")
                nc.sync.dma_start(out=t, in_=W_res.ap()[128 * j : 128 * (j + 1), :])
                tb = singles.tile([128, 4 * H], bf16, tag=f"wnb{j}")
                nc.vector.tensor_copy(tb, t)
                for f in range(8):
                    pt = ps_tr.tile([128, 128], bf16, tag="tr")
                    nc.tensor.transpose(
                        pt, tb[:, 128 * f : 128 * (f + 1)], identity_bf
                    )
                    nc.any.tensor_copy(
                        out=WT[f][:, 128 * j : 128 * (j + 1)], in_=pt
                    )

            def frontend(b):
                st = {}
                # ---- load ----
                C_nat = []
                Q_nat = []
                for k in range(KH):
                    t = sbig.tile([128, LC], f32r, tag=f"cnat{k}")
                    nc.sync.dma_start(
                        out=t,
                        in_=C.ap()[b, 128 * k : 128 * (k + 1), :].bitcast(f32r),
                    )
                    C_nat.append(t)
                    tq = sb.tile([128, LQ], f32r, tag=f"qnat{k}")
                    nc.sync.dma_start(
                        out=tq,
                        in_=Q.ap()[b, 128 * k : 128 * (k + 1), :].bitcast(f32r),
                    )
                    Q_nat.append(tq)

                # ---- bf16 copies + DMA-xbar transposes ----
                C_bf = []
                Q_bf = []
                for k in range(KH):
                    cb = sbig.tile([128, LC], bf16, tag=f"cbf{k}")
                    nc.vector.tensor_copy(cb, C_nat[k].bitcast(f32))
                    C_bf.append(cb)
                    qb = sb.tile([128, LQ], bf16, tag=f"qbf{k}")
                    nc.vector.tensor_copy(qb, Q_nat[k].bitcast(f32))
                    Q_bf.append(qb)

                # CtT[i][p, h] = C^T[128*i + p, h];  QT[j][p, h] = Q^T[128*j + p, h]
                CtT = []
                for i in range(NLT):
                    t_ct = sb1.tile([128, H], bf16, tag=f"ctt{i}")
                    for k in range(KH):
                        pt = ps_tr.tile([128, 128], bf16, tag="tr")
                        nc.tensor.transpose(
                            pt, C_bf[k][:, 128 * i : 128 * (i + 1)], identity_bf
                        )
                        nc.any.tensor_copy(
                            out=t_ct[:, 128 * k : 128 * (k + 1)], in_=pt
                        )
                    CtT.append(t_ct)
                QT = []
                for j in range(NMT):
                    t_qt = sb1.tile([128, H], bf16, tag=f"qt{j}")
                    for k in range(KH):
                        pt = ps_tr.tile([128, 128], bf16, tag="tr")
                        nc.tensor.transpose(
                            pt, Q_bf[k][:, 128 * j : 128 * (j + 1)], identity_bf
                        )
                        nc.any.tensor_copy(
                            out=t_qt[:, 128 * k : 128 * (k + 1)], in_=pt
                        )
                    QT.append(t_qt)

                # ---- affine-augmented operands ----
                CA = []
                QA = []
                for k in range(KH):
                    t = sbig.tile([128, LC], f32r, tag=f"ca{k}")
                    nc.vector.tensor_scalar(
                        out=t,
                        in0=C_nat[k],
                        scalar1=w3_col[:, k : k + 1],
                        scalar2=w2_col[:, k : k + 1].bitcast(f32),
                        op0=OP.mult,
                        op1=OP.add,
                    )
                    CA.append(t)
                    tq = sb.tile([128, LQ], f32r, tag=f"qa{k}")
                    nc.vector.tensor_scalar(
                        out=tq,
                        in0=Q_nat[k],
                        scalar1=w3_col[:, k : k + 1],
                        scalar2=w1_col[:, k : k + 1].bitcast(f32),
                        op0=OP.mult,
                        op1=OP.add,
                    )
                    QA.append(tq)

                # ---- r,q bias rows -> per-partition columns (DRAM bounce) ----
                r_row = sb.tile([1, LC], f32, tag="rrow")
                for c in range(2):
                    ps_r = ps_z.tile([1, 512], f32, tag="z")
                    for k in range(KH):
                        mm(
                            ps_r,
                            w1_col[:, k : k + 1],
                            C_nat[k][:, 512 * c : 512 * (c + 1)],
                            (k == 0),
                            (k == KH - 1),
                        )
                    nc.any.tensor_copy(
                        out=r_row[:, 512 * c : 512 * (c + 1)], in_=ps_r
                    )
                r_dram = dr.tile([1, LC], f32, tag="rd")
                nc.sync.dma_start(out=r_dram, in_=r_row)
                r_col = sb.tile([128, NLT], f32, tag="rcol")
                nc.sync.dma_start(
                    out=r_col,
                    in_=r_dram.rearrange("1 (i p) -> p i", i=NLT, p=128),
                )

                ps_q = ps_z.tile([1, LQ], f32, tag="z")
                for k in range(KH):
                    mm(
                        ps_q,
                        w2_col[:, k : k + 1],
                        Q_nat[k],
                        (k == 0),
                        (k == KH - 1),
                    )
                q_row = sb.tile([1, LQ], f32, tag="qrow")
                nc.any.tensor_copy(out=q_row, in_=ps_q)
                q_dram = dr.tile([1, LQ], f32, tag="qd")
                nc.sync.dma_start(out=q_dram, in_=q_row)
                q_col = sb.tile([128, NMT], f32, tag="qcol")
                nc.sync.dma_start(
                    out=q_col,
                    in_=q_dram.rearrange("1 (i p) -> p i", i=NMT, p=128),
                )

                st.update(
                    C_nat=C_nat, Q_nat=Q_nat, C_bf=C_bf, CtT=CtT, QT=QT,
                    CA=CA, QA=QA, r_col=r_col, q_col=q_col,
                )
                return st

            def backend(b, st):
                C_nat = st["C_nat"]; Q_nat = st["Q_nat"]; C_bf = st["C_bf"]
                CtT = st["CtT"]; QT = st["QT"]; CA = st["CA"]; QA = st["QA"]
                r_col = st["r_col"]; q_col = st["q_col"]

                if True:
                    # ---- Z in [l, m] layout + rowsums rho ----
                    rho_col = sb.tile([128, NLT], f32, tag="rho")
                    E_lm = []
                    for i in range(NLT):
                        pz = ps_z.tile([128, LQ], f32, tag="z")
                        for k in range(KH):
                            mm(
                                pz,
                                CA[k][:, 128 * i : 128 * (i + 1)],
                                Q_nat[k],
                                (k == 0),
                                (k == KH - 1),
                            )
                        e = sb1.tile([128, LQ], bf16, tag=f"elm{i}")
                        nc.scalar.activation(
                            out=e,
                            in_=pz,
                            func=AF.Exp,
                            bias=r_col[:, i : i + 1],
                            accum_out=rho_col[:, i : i + 1],
                        )
                        E_lm.append(e)

                    # ---- Z in [m, l] layout + colsums kappa ----
                    kap_col = sb.tile([128, NMT], f32, tag="kap")
                    E_ml = []
                    for j in range(NMT):
                        pzt = ps_big.tile([128, LC], f32, tag="big")
                        for k in range(KH):
                            for c in range(2):
                                mm(
                                    pzt[:, 512 * c : 512 * (c + 1)],
                                    QA[k][:, 128 * j : 128 * (j + 1)],
                                    C_nat[k][:, 512 * c : 512 * (c + 1)],
                                    (k == 0),
                                    (k == KH - 1),
                                )
                        e = sbig1.tile([128, LC], bf16, tag=f"eml{j}")
                        nc.scalar.activation(
                            out=e,
                            in_=pzt,
                            func=AF.Exp,
                            bias=q_col[:, j : j + 1],
                            accum_out=kap_col[:, j : j + 1],
                        )
                        E_ml.append(e)

                    # ---- reciprocals ----
                    rho_inv = sb.tile([128, NLT], f32, tag="rhoi")
                    nc.vector.reciprocal(rho_inv, rho_col)
                    kap_inv = sb.tile([128, NMT], f32, tag="kapi")
                    nc.vector.reciprocal(kap_inv, kap_col)

                    # rho_inv -> bf16 row layout, broadcast to all partitions
                    rho_inv_bf = sb.tile([128, NLT], bf16, tag="rhoib")
                    nc.vector.tensor_copy(rho_inv_bf, rho_inv)
                    ri_dram = dr.tile([1, LC], bf16, tag="rid")
                    nc.sync.dma_start(
                        out=ri_dram.rearrange("1 (i p) -> p i", i=NLT, p=128),
                        in_=rho_inv_bf,
                    )
                    ri_bc = sbig1.tile([128, LC], bf16, tag="ribc")
                    bc_src = bass.AP(
                        tensor=ri_dram.tensor,
                        offset=ri_dram.offset,
                        ap=[[0, 128], [1, LC]],
                    )
                    nc.sync.dma_start(out=ri_bc, in_=bc_src)

                    # ---- P^T = Z^T / rho  (row-softmax, transposed layout) ----
                    P_ml = []
                    for j in range(NMT):
                        t = sbig1.tile([128, LC], bf16, tag=f"pml{j}")
                        nc.vector.tensor_mul(t, E_ml[j], ri_bc)
                        P_ml.append(t)

                    # ---- T = S_col^T @ Ct   [m, h] ----
                    T_nat = []
                    for j in range(NMT):
                        pT = ps_z.tile([128, H], f32, tag="z")
                        for i in range(NLT):
                            mmb(
                                pT,
                                E_lm[i][:, 128 * j : 128 * (j + 1)],
                                CtT[i],
                                (i == 0),
                                (i == NLT - 1),
                            )
                        t = sb1.tile([128, H], bf16, tag=f"tn{j}")
                        nc.vector.tensor_scalar_mul(t, pT, kap_inv[:, j : j + 1])
                        T_nat.append(t)

                    # ---- A^T and Bv^T  [h, l] ----
                    A_T = []
                    Bv_T = []
                    for t_i in range(KH):
                        pA = ps_big.tile([128, LC], f32, tag="big")
                        for k in range(NMT):
                            for c in range(2):
                                mmb(
                                    pA[:, 512 * c : 512 * (c + 1)],
                                    QT[k][:, 128 * t_i : 128 * (t_i + 1)],
                                    P_ml[k][:, 512 * c : 512 * (c + 1)],
                                    (k == 0),
                                    (k == NMT - 1),
                                )
                        a = sbig1.tile([128, LC], bf16, tag=f"at{t_i}")
                        nc.any.tensor_copy(out=a, in_=pA)
                        A_T.append(a)
                    for t_i in range(KH):
                        pB = ps_big.tile([128, LC], f32, tag="big")
                        for k in range(NMT):
                            for c in range(2):
                                mmb(
                                    pB[:, 512 * c : 512 * (c + 1)],
                                    T_nat[k][:, 128 * t_i : 128 * (t_i + 1)],
                                    P_ml[k][:, 512 * c : 512 * (c + 1)],
                                    (k == 0),
                                    (k == NMT - 1),
                                )
                        bv = sbig1.tile([128, LC], bf16, tag=f"bvt{t_i}")
                        nc.any.tensor_copy(out=bv, in_=pB)
                        Bv_T.append(bv)

                    # ---- products ----
                    CA1 = []
                    CB1 = []
                    for t_i in range(KH):
                        p1 = sbig1.tile([128, LC], bf16, tag=f"ca1{t_i}")
                        nc.vector.tensor_mul(p1, C_bf[t_i], A_T[t_i])
                        CA1.append(p1)
                        p2 = sbig1.tile([128, LC], bf16, tag=f"cb1{t_i}")
                        nc.vector.tensor_mul(p2, C_bf[t_i], Bv_T[t_i])
                        CB1.append(p2)

                    # ---- final matmul + relu + store ----
                    blocks = [
                        C_bf[0],
                        C_bf[1],
                        A_T[0],
                        A_T[1],
                        CA1[0],
                        CA1[1],
                        CB1[0],
                        CB1[1],
                    ]
                    for t_i in range(KH):
                        po = ps_big.tile([128, LC], f32, tag="big")
                        for f in range(8):
                            for c in range(2):
                                mmb(
                                    po[:, 512 * c : 512 * (c + 1)],
                                    WT[f][:, 128 * t_i : 128 * (t_i + 1)],
                                    blocks[f][:, 512 * c : 512 * (c + 1)],
                                    (f == 0),
                                    (f == 7),
                                )
                        o = sbig.tile([128, LC], f32, tag=f"osb{t_i}")
                        nc.scalar.activation(
                            out=o,
                            in_=po,
                            func=AF.Relu,
                            bias=b_col[:, t_i : t_i + 1],
                        )
                        nc.sync.dma_start(
                            out=out.ap()[b, 128 * t_i : 128 * (t_i + 1), :], in_=o
                        )

            def body(iv=None):
                st_prev = None
                for b in range(BB):
                    st = frontend(b)
                    if st_prev is not None:
                        backend(b - 1, st_prev)
                    st_prev = st
                backend(BB - 1, st_prev)

            if reps == 1:
                body()
            else:
                with tc.For_i(0, reps, 1) as iv:
                    body(iv)

    nc.compile()
    return nc


def _get_nc(reps: int = 1):
    key = ("nc", reps)
    if key not in _CACHE:
        _CACHE[key] = _build(reps)
    return _CACHE[key]


def kernel(C, Q, cmask, qmask, w, W_res, b_res, _reps: int = 1, _want_res: bool = False):
    from concourse.bass_utils import run_bass_kernel_spmd

    nc = _get_nc(_reps)

    C = np.ascontiguousarray(C, dtype=np.float32)
    Q = np.ascontiguousarray(Q, dtype=np.float32)
    w = np.ascontiguousarray(w, dtype=np.float32)
    W_res = np.ascontiguousarray(W_res, dtype=np.float32)
    b_res = np.ascontiguousarray(b_res, dtype=np.float32)

    in_maps = []
    for i in range(N_CORES):
        sl = slice(i * BB, (i + 1) * BB)
        in_maps.append(
            {"C": C[sl], "Q": Q[sl], "w": w, "W_res": W_res, "b_res": b_res}
        )

    res = run_bass_kernel_spmd(nc, in_maps, core_ids=list(range(N_CORES)))
    out = np.concatenate([res.results[i]["out"] for i in range(N_CORES)], axis=0)
    if _want_res:
        return out, res
    return out



# revision 40
# speedup vs baseline: 15.3423x; 15.3423x over previous
"""CQAttention Trainium2 kernel — data-parallel over batch across 8 NeuronCores.

Problem shapes (hardcoded): B=32, H=256, Lc=1024, Lq=256.
Each core processes B/8 = 4 batches.

Math (per batch, with all-ones masks — guaranteed by the problem spec):
  Ct = C^T [Lc,H], Qt = Q^T [Lq,H]
  S[l,m] = r[l] + q[m] + (Ct[l]*w3)@Qt[m]   (r = Ct@w1, q = Qt@w2)
  Z = exp(S) serves BOTH softmaxes:
    S_row = Z / rowsum(Z),  S_col = Z / colsum(Z)
  A  = S_row @ Qt
  Bv = S_row @ (S_col^T @ Ct)      (factored: avoids the Lc x Lc product)
  out = relu([Ct, A, Ct*A, Ct*Bv] @ W_res^T + b_res)^T  -> [H, Lc]

Implementation notes (fully on-chip; no DRAM bounces, no DMA on the
critical path):
  - Logit matmuls run in bf16 (PE 1 cyc/col; accumulation is fp32 in
    PSUM). Everything downstream of exp is bf16. rel err ~4e-3 vs the
    2e-2 gate.
  - Z is computed in both [l,m] and [m,l] layouts (each is needed as a
    contraction operand with l resp. m on partitions); exp's accum_out
    yields the row/col softmax sums for free.
  - The r[l]/q[m] bias columns for exp are produced on-chip: r/q rows
    come from w1.C / w2.Q matmuls, then ten tiny N=1 matmuls
    (row_chunk^T @ [1.0]) flip them into one [128,10] PSUM column set.
  - 1/rho is broadcast on-chip: rho column [128,8] -> PE transpose ->
    [8,128] rows -> eight K=8 selector matmuls -> ri_bc [128,Lc]. The
    A/Bv PSUM drains multiply by ri_bc directly (the scaled attention
    matrix never materializes), so no DMA round-trip gates the batch.
  - PE transposes drain 4-at-a-time through [128,512] PSUM tiles into
    wide contiguous tiles (CtT_all/QT_all/WT_all).
  - Engine split: ACT = exp/relu/T-scale; Vector = PSUM drains, casts,
    products; GpSimd (no PSUM port) = SBUF-only affine operands.
    Never write a tile in sub-ranges from a compute engine and read it
    from PE LDWEIGHTS — Tile misses that dependency (observed miscompute).
  - Two HWDGE queues: SP carries the per-batch C/Q loads (prefetched two
    batches ahead) + output stores; the Activation queue carries all
    one-time loads (W_res chunks, b_col, sel8) so they never delay
    batch-0 inputs.
  - Emission is software-pipelined: loads(b+2), frontend(b+1), backend(b).
"""

import numpy as np

_CACHE = {}

B_FULL = 32
N_CORES = 8
BB = B_FULL // N_CORES  # batches per core = 4
H = 256
LC = 1024
LQ = 256


def _build(reps: int = 1):
    from contextlib import ExitStack

    import concourse.bass as bass
    import concourse.tile as tile
    from concourse import bacc, mybir
    from concourse.masks import make_identity

    f32 = mybir.dt.float32
    f32r = mybir.dt.float32r
    bf16 = mybir.dt.bfloat16
    AF = mybir.ActivationFunctionType
    OP = mybir.AluOpType

    nc = bacc.Bacc("TRN2", target_bir_lowering=False, debug=False)

    def mm(out, lhsT, rhs, start, stop):
        nc.tensor.matmul(
            out,
            lhsT=lhsT.bitcast(f32r),
            rhs=rhs.bitcast(f32r),
            start=start,
            stop=stop,
        )

    def mmb(out, lhsT, rhs, start, stop):
        nc.tensor.matmul(out, lhsT=lhsT, rhs=rhs, start=start, stop=stop)

    C = nc.dram_tensor("C", [BB, H, LC], f32, kind="ExternalInput")
    Q = nc.dram_tensor("Q", [BB, H, LQ], f32, kind="ExternalInput")
    w = nc.dram_tensor("w", [3 * H], f32, kind="ExternalInput")
    W_res = nc.dram_tensor("W_res", [H, 4 * H], f32, kind="ExternalInput")
    b_res = nc.dram_tensor("b_res", [H], f32, kind="ExternalInput")
    out = nc.dram_tensor("out", [BB, H, LC], f32, kind="ExternalOutput")

    KH = H // 128  # 2 h-chunks
    NLT = LC // 128  # 8 l-tiles
    NMT = LQ // 128  # 2 m-tiles

    with tile.TileContext(nc) as tc:
        with ExitStack() as ctx:
            singles = ctx.enter_context(tc.tile_pool(name="singles", bufs=1))
            sb = ctx.enter_context(tc.tile_pool(name="sb", bufs=2))
            sb1 = ctx.enter_context(tc.tile_pool(name="sb1", bufs=3))
            sbig = ctx.enter_context(tc.tile_pool(name="sbig", bufs=2))
            sbig1 = ctx.enter_context(tc.tile_pool(name="sbig1", bufs=3))
            ps_tr = ctx.enter_context(
                tc.tile_pool(name="ps_tr", bufs=2, space="PSUM")
            )
            ps_z = ctx.enter_context(
                tc.tile_pool(name="ps_z", bufs=2, space="PSUM")
            )
            ps_big = ctx.enter_context(
                tc.tile_pool(name="ps_big", bufs=2, space="PSUM")
            )

            # ---- one-time constants ----
            identity_bf = singles.tile([128, 128], bf16)
            make_identity(nc, identity_bf)


            w1_col = singles.tile([128, KH], f32r)
            w2_col = singles.tile([128, KH], f32r)
            w3_col = singles.tile([128, KH], f32)
            nc.sync.dma_start(
                out=w1_col,
                in_=w.ap()[0:H].rearrange("(i p) -> p i", i=KH, p=128).bitcast(f32r),
            )
            nc.sync.dma_start(
                out=w2_col,
                in_=w.ap()[H : 2 * H]
                .rearrange("(i p) -> p i", i=KH, p=128)
                .bitcast(f32r),
            )
            nc.sync.dma_start(
                out=w3_col,
                in_=w.ap()[2 * H : 3 * H].rearrange("(i p) -> p i", i=KH, p=128),
            )
            # W_res^T (bf16): WT_all[:, 256*f + 128*j : +128] = W_res[128j:128(j+1), 128f:128(f+1)]^T
            # Loaded in four [128, 512] chunks (separate tiles, full-tile
            # writes) so the load->cast->transpose chain pipelines at startup.
            WT_all = singles.tile([128, 4 * H * KH], bf16)
            wn = {}
            for hh in range(2):
                for j in range(KH):
                    t = singles.tile([128, 512], f32, tag=f"wn{j}_{hh}")
                    nc.scalar.dma_start(
                        out=t,
                        in_=W_res.ap()[
                            128 * j : 128 * (j + 1), 512 * hh : 512 * (hh + 1)
                        ],
                    )
                    tb = singles.tile([128, 512], bf16, tag=f"wnb{j}_{hh}")
                    eng = nc.vector if (j + hh) % 2 == 0 else nc.gpsimd
                    eng.tensor_copy(tb, t)
                    wn[(j, hh)] = tb
            b_col = singles.tile([128, KH], f32)
            nc.scalar.dma_start(
                out=b_col, in_=b_res.ap().rearrange("(i p) -> p i", i=KH, p=128)
            )

            # sel8[k, 128*i + q] = (k == i): K=8 selector for broadcasting row i
            # of an [8, 128] tile to all 128 output partitions via one matmul.
            # Compute engines can't address base partition i>0, so the ones
            # rows are planted by tiny one-time SBUF->SBUF DMAs.
            ones_bf_row = singles.tile([1, 128], bf16)
            nc.gpsimd.memset(ones_bf_row, 1.0)
            sel8 = singles.tile([NLT, NLT * 128], bf16)
            nc.gpsimd.memset(sel8, 0.0)
            for i in range(NLT):
                nc.scalar.dma_start(
                    out=sel8[i : i + 1, 128 * i : 128 * (i + 1)], in_=ones_bf_row
                )

            seq = [(f, j) for f in range(8) for j in range(KH)]
            for g in range(4):
                pt = ps_tr.tile([128, 512], bf16, tag="tr")
                for s in range(4):
                    f, j = seq[4 * g + s]
                    nc.tensor.transpose(
                        pt[:, 128 * s : 128 * (s + 1)],
                        wn[(j, f // 4)][:, 128 * (f % 4) : 128 * (f % 4 + 1)],
                        identity_bf,
                    )
                nc.vector.tensor_copy(
                    out=WT_all[:, 512 * g : 512 * (g + 1)], in_=pt
                )

            def wt(f, t_i):
                return WT_all[:, 256 * f + 128 * t_i : 256 * f + 128 * (t_i + 1)]

            def loads(b):
                C_nat = []
                Q_nat = []
                for k in range(KH):
                    t = sbig.tile([128, LC], f32r, tag=f"cnat{k}", bufs=3)
                    nc.sync.dma_start(
                        out=t,
                        in_=C.ap()[b, 128 * k : 128 * (k + 1), :].bitcast(f32r),
                    )
                    C_nat.append(t)
                    tq = sb.tile([128, LQ], f32r, tag=f"qnat{k}", bufs=3)
                    nc.sync.dma_start(
                        out=tq,
                        in_=Q.ap()[b, 128 * k : 128 * (k + 1), :].bitcast(f32r),
                    )
                    Q_nat.append(tq)
                return C_nat, Q_nat

            def frontend(b, ld):
                st = {}
                C_nat, Q_nat = ld

                # ---- r_row = w1.C  [1, LC],  q_row = w2.Q  [1, LQ] (bf16) ----
                r_row = sb.tile([1, LC], bf16, tag="rrow")
                for c in range(2):
                    ps_r = ps_tr.tile([1, 512], f32, tag="tr")
                    for k in range(KH):
                        mm(
                            ps_r,
                            w1_col[:, k : k + 1],
                            C_nat[k][:, 512 * c : 512 * (c + 1)],
                            (k == 0),
                            (k == KH - 1),
                        )
                    nc.vector.tensor_copy(
                        out=r_row[:, 512 * c : 512 * (c + 1)], in_=ps_r
                    )
                ps_q = ps_tr.tile([1, LQ], f32, tag="tr")
                for k in range(KH):
                    mm(
                        ps_q,
                        w2_col[:, k : k + 1],
                        Q_nat[k],
                        (k == 0),
                        (k == KH - 1),
                    )
                q_row = sb.tile([1, LQ], bf16, tag="qrow")
                nc.vector.tensor_copy(out=q_row, in_=ps_q)

                # ---- flip r/q rows into per-partition bias columns via ten
                # ---- tiny N=1 matmuls (lhsT^T @ [1.0]), batched in one PSUM ----
                ptr_rq = ps_tr.tile([128, 16], f32, tag="tr")
                for i in range(NLT):
                    mmb(
                        ptr_rq[:, i : i + 1],
                        r_row[:, 128 * i : 128 * (i + 1)],
                        identity_bf[0:1, 0:1],
                        True,
                        True,
                    )
                for j in range(NMT):
                    mmb(
                        ptr_rq[:, NLT + j : NLT + j + 1],
                        q_row[:, 128 * j : 128 * (j + 1)],
                        identity_bf[0:1, 0:1],
                        True,
                        True,
                    )
                rq_col = sb.tile([128, NLT + NMT], f32, tag="rqcol")
                nc.vector.tensor_copy(rq_col, ptr_rq[:, 0 : NLT + NMT])

                # ---- bf16 copies ----
                C_bf = []
                Q_bf = []
                for k in range(KH):
                    cb = sbig.tile([128, LC], bf16, tag=f"cbf{k}")
                    nc.vector.tensor_copy(cb, C_nat[k].bitcast(f32))
                    C_bf.append(cb)
                    qb = sb.tile([128, LQ], bf16, tag=f"qbf{k}")
                    nc.vector.tensor_copy(qb, Q_nat[k].bitcast(f32))
                    Q_bf.append(qb)

                # ---- PE transposes, batched drains ----
                # CtT_all[:, 256*i + 128*k : +128] = C^T l-tile i, h-chunk k
                CtT_all = sb1.tile([128, 2 * H * NLT // 2], bf16, tag="ctt")
                cseq = [(i, k) for i in range(NLT) for k in range(KH)]
                for g in range(4):
                    pt = ps_tr.tile([128, 512], bf16, tag="tr")
                    for s in range(4):
                        i, k = cseq[4 * g + s]
                        nc.tensor.transpose(
                            pt[:, 128 * s : 128 * (s + 1)],
                            C_bf[k][:, 128 * i : 128 * (i + 1)],
                            identity_bf,
                        )
                    nc.vector.tensor_copy(
                        out=CtT_all[:, 512 * g : 512 * (g + 1)], in_=pt
                    )
                QT_all = sb.tile([128, H * NMT], bf16, tag="qt")
                qseq = [(j, k) for j in range(NMT) for k in range(KH)]
                pt = ps_tr.tile([128, 512], bf16, tag="tr")
                for s in range(4):
                    j, k = qseq[s]
                    nc.tensor.transpose(
                        pt[:, 128 * s : 128 * (s + 1)],
                        Q_bf[k][:, 128 * j : 128 * (j + 1)],
                        identity_bf,
                    )
                nc.vector.tensor_copy(out=QT_all, in_=pt)

                # ---- affine-augmented operands (GpSimd: SBUF-only) ----
                # CA = C*w3 + w2 so CA^T@Q = dot + q[m]; QA = Q*w3 + w1 so
                # QA^T@C = dot + r[l]. bf16: the PE runs bf16 at 1 cyc/col
                # vs ~1.1-1.4 for fp32r, and accumulation stays fp32.
                CA = []
                QA = []
                for k in range(KH):
                    t = sbig.tile([128, LC], bf16, tag=f"ca{k}")
                    eng = nc.gpsimd if k == 0 else nc.vector
                    eng.tensor_scalar(
                        out=t,
                        in0=C_nat[k],
                        scalar1=w3_col[:, k : k + 1],
                        scalar2=w2_col[:, k : k + 1].bitcast(f32),
                        op0=OP.mult,
                        op1=OP.add,
                    )
                    CA.append(t)
                    tq = sb.tile([128, LQ], bf16, tag=f"qa{k}")
                    nc.vector.tensor_scalar(
                        out=tq,
                        in0=Q_nat[k],
                        scalar1=w3_col[:, k : k + 1],
                        scalar2=w1_col[:, k : k + 1].bitcast(f32),
                        op0=OP.mult,
                        op1=OP.add,
                    )
                    QA.append(tq)

                st.update(
                    C_nat=C_nat, Q_nat=Q_nat, C_bf=C_bf, Q_bf=Q_bf,
                    CtT_all=CtT_all, QT_all=QT_all, CA=CA, QA=QA,
                    rq_col=rq_col,
                )
                return st

            def backend(b, st):
                C_nat = st["C_nat"]; Q_nat = st["Q_nat"]; C_bf = st["C_bf"]
                Q_bf = st["Q_bf"]; CtT_all = st["CtT_all"]; QT_all = st["QT_all"]
                CA = st["CA"]; QA = st["QA"]; rq_col = st["rq_col"]

                # ---- Z in [l, m] layout + rowsums rho ----
                # S = (C*w3)^T Q + r x 1 + 1 x q ; the rank-1 terms enter the
                # PSUM accumulation directly (fp32r), no exp-bias needed.
                rho_col = sb.tile([128, NLT], f32, tag="rho")
                E_lm = []
                for i in range(NLT):
                    pz = ps_z.tile([128, LQ], f32, tag="z")
                    for k in range(KH):
                        mmb(
                            pz,
                            CA[k][:, 128 * i : 128 * (i + 1)],
                            Q_bf[k],
                            (k == 0),
                            (k == KH - 1),
                        )
                    e = sb1.tile([128, LQ], bf16, tag=f"elm{i}")
                    nc.scalar.activation(
                        out=e,
                        in_=pz,
                        func=AF.Exp,
                        bias=rq_col[:, i : i + 1],
                        accum_out=rho_col[:, i : i + 1],
                    )
                    E_lm.append(e)

                # ---- Z in [m, l] layout + colsums kappa ----
                kap_col = sb.tile([128, NMT], f32, tag="kap")
                E_ml = []
                for j in range(NMT):
                    pzt = ps_big.tile([128, LC], f32, tag="big")
                    for c in range(2):
                        sl = slice(512 * c, 512 * (c + 1))
                        for k in range(KH):
                            mmb(
                                pzt[:, sl],
                                QA[k][:, 128 * j : 128 * (j + 1)],
                                C_bf[k][:, sl],
                                (k == 0),
                                (k == KH - 1),
                            )
                    e = sbig1.tile([128, LC], bf16, tag=f"eml{j}")
                    nc.scalar.activation(
                        out=e,
                        in_=pzt,
                        func=AF.Exp,
                        bias=rq_col[:, NLT + j : NLT + j + 1],
                        accum_out=kap_col[:, j : j + 1],
                    )
                    E_ml.append(e)

                # ---- reciprocals ----
                kap_inv = sb.tile([128, NMT], f32, tag="kapi")
                nc.vector.reciprocal(kap_inv, kap_col)

                # ---- T = S_col^T @ Ct   [m, h] ----
                T_nat = []
                for j in range(NMT):
                    pT = ps_z.tile([128, H], f32, tag="z")
                    for i in range(NLT):
                        mmb(
                            pT,
                            E_lm[i][:, 128 * j : 128 * (j + 1)],
                            CtT_all[:, 256 * i : 256 * (i + 1)],
                            (i == 0),
                            (i == NLT - 1),
                        )
                    t = sb1.tile([128, H], bf16, tag=f"tn{j}")
                    nc.scalar.activation(
                        out=t, in_=pT, func=AF.Copy, scale=kap_inv[:, j : j + 1]
                    )
                    T_nat.append(t)

                # ---- 1/rho broadcast: column -> rows -> [128, LC] ----
                rho_inv = sb.tile([128, NLT], f32, tag="rhoi")
                nc.vector.reciprocal(rho_inv, rho_col)
                rho_inv_bf = sb.tile([128, NLT], bf16, tag="rhoib")
                nc.gpsimd.tensor_copy(rho_inv_bf, rho_inv)
                ptr = ps_tr.tile([NLT, 128], bf16, tag="tr")
                nc.tensor.transpose(ptr, rho_inv_bf, identity_bf)
                rho_rows = sb.tile([NLT, 128], bf16, tag="rrows")
                nc.vector.tensor_copy(rho_rows, ptr)
                ri_bc = sbig1.tile([128, LC], bf16, tag="ribc")
                for half in range(2):
                    pri = ps_tr.tile([128, 512], f32, tag="tr")
                    for s in range(4):
                        i = 4 * half + s
                        mmb(
                            pri[:, 128 * s : 128 * (s + 1)],
                            sel8[:, 128 * i : 128 * (i + 1)],
                            rho_rows,
                            True,
                            True,
                        )
                    nc.vector.tensor_copy(
                        out=ri_bc[:, 512 * half : 512 * (half + 1)], in_=pri
                    )

                # ---- A^T and Bv^T  [h, l]: matmuls on unscaled E_ml, the
                # ---- PSUM drain multiplies in 1/rho[l] ----
                A_T = []
                Bv_T = []
                for t_i in range(KH):
                    pA = ps_big.tile([128, LC], f32, tag="big")
                    for k in range(NMT):
                        for c in range(2):
                            sl = slice(512 * c, 512 * (c + 1))
                            mmb(
                                pA[:, sl],
                                QT_all[:, 256 * k + 128 * t_i : 256 * k + 128 * (t_i + 1)],
                                E_ml[k][:, sl],
                                (k == 0),
                                (k == NMT - 1),
                            )
                    a = sbig1.tile([128, LC], bf16, tag=f"at{t_i}")
                    nc.vector.tensor_mul(a, pA, ri_bc)
                    A_T.append(a)
                for t_i in range(KH):
                    pB = ps_big.tile([128, LC], f32, tag="big")
                    for k in range(NMT):
                        for c in range(2):
                            sl = slice(512 * c, 512 * (c + 1))
                            mmb(
                                pB[:, sl],
                                T_nat[k][:, 128 * t_i : 128 * (t_i + 1)],
                                E_ml[k][:, sl],
                                (k == 0),
                                (k == NMT - 1),
                            )
                    bv = sbig1.tile([128, LC], bf16, tag=f"bvt{t_i}")
                    nc.vector.tensor_mul(bv, pB, ri_bc)
                    Bv_T.append(bv)

                # ---- products (GpSimd: SBUF-only) ----
                CA1 = []
                CB1 = []
                for t_i in range(KH):
                    p1 = sbig1.tile([128, LC], bf16, tag=f"ca1{t_i}")
                    nc.vector.tensor_tensor(
                        out=p1, in0=C_bf[t_i], in1=A_T[t_i], op=OP.mult
                    )
                    CA1.append(p1)
                    p2 = sbig1.tile([128, LC], bf16, tag=f"cb1{t_i}")
                    nc.vector.tensor_tensor(
                        out=p2, in0=C_bf[t_i], in1=Bv_T[t_i], op=OP.mult
                    )
                    CB1.append(p2)

                # ---- final matmul + relu + store ----
                blocks = [
                    C_bf[0],
                    C_bf[1],
                    A_T[0],
                    A_T[1],
                    CA1[0],
                    CA1[1],
                    CB1[0],
                    CB1[1],
                ]
                for t_i in range(KH):
                    po = ps_big.tile([128, LC], f32, tag="big")
                    for f in range(8):
                        for c in range(2):
                            sl = slice(512 * c, 512 * (c + 1))
                            mmb(
                                po[:, sl],
                                wt(f, t_i),
                                blocks[f][:, sl],
                                (f == 0),
                                (f == 7),
                            )
                    o = sbig.tile([128, LC], f32, tag=f"osb{t_i}")
                    nc.scalar.activation(
                        out=o,
                        in_=po,
                        func=AF.Relu,
                        bias=b_col[:, t_i : t_i + 1],
                    )
                    nc.sync.dma_start(
                        out=out.ap()[b, 128 * t_i : 128 * (t_i + 1), :], in_=o
                    )

            def body(iv=None):
                ld = {0: loads(0), 1: loads(1)}
                st_prev = None
                for b in range(BB):
                    if b + 2 < BB:
                        ld[b + 2] = loads(b + 2)
                    st = frontend(b, ld.pop(b))
                    if st_prev is not None:
                        backend(b - 1, st_prev)
                    st_prev = st
                backend(BB - 1, st_prev)

            if reps == 1:
                body()
            else:
                with tc.For_i(0, reps, 1) as iv:
                    body(iv)

    nc.compile()
    return nc


def _get_nc(reps: int = 1):
    key = ("nc", reps)
    if key not in _CACHE:
        _CACHE[key] = _build(reps)
    return _CACHE[key]


def kernel(C, Q, cmask, qmask, w, W_res, b_res, _reps: int = 1, _want_res: bool = False,
           _trace: bool = False, _tmpdir: str | None = None):
    from concourse.bass_utils import run_bass_kernel_spmd

    nc = _get_nc(_reps)

    C = np.ascontiguousarray(C, dtype=np.float32)
    Q = np.ascontiguousarray(Q, dtype=np.float32)
    w = np.ascontiguousarray(w, dtype=np.float32)
    W_res = np.ascontiguousarray(W_res, dtype=np.float32)
    b_res = np.ascontiguousarray(b_res, dtype=np.float32)

    in_maps = []
    for i in range(N_CORES):
        sl = slice(i * BB, (i + 1) * BB)
        in_maps.append(
            {"C": C[sl], "Q": Q[sl], "w": w, "W_res": W_res, "b_res": b_res}
        )

    res = run_bass_kernel_spmd(
        nc, in_maps, core_ids=list(range(N_CORES)), trace=_trace, tmpdir=_tmpdir
    )
    out = np.concatenate([res.results[i]["out"] for i in range(N_CORES)], axis=0)
    if _want_res:
        return out, res
    return out
